# revision 1
# baseline (speedup 1.0000x reference)
"""Trainium2 Bass kernel for nn_AttackLoss (nms_detection).

Computes, for O=2048 ground-truth boxes vs D=8732 detections:
    best[o]  = max IoU over same-label detections of object o
    loss     = sum(has_match * (1 - best)) / sum(has_match)

Fast path: classes are sorted by det count and sharded whole across the
8 cores x 3 slots (one class per slot; no label masking inside a slot).
Objects of the slot's class sit on SBUF partitions (corners as fp32
per-partition scalar columns), its detections along the free axis as
fp16 rows broadcast across partitions by stride-0 DMA reads straight
from DRAM. The IoU max uses the monotone identity iou = q/(1-q) with
q = inter/(ao+ad): max_d iou = g(max_d q), so no per-pair union-minus-
intersection is needed and the host does the per-object q->iou map.
Per-slot passes (fp16 coords feed the DVE's 2x/4x fast modes; the
division tail stays fp32 for the bit-trick reciprocal):

    DVE: Bx = max(dx1, ox1); Mx = min(dx2, ox2)   tensor_scalar (4x)
    DVE: U = Mx - Bx                              tensor_tensor (2x)
    DVE: Urn = -max(U, 0)                         tensor_scalar (4x)
    DVE: By, My, V = My - By                      same for y
    DVE: interN = Urn * V                         tensor_tensor (2x)
    DVE: s = ad + ao  (fp32 out)                  tensor_scalar
    DVE: rec ~ 1/s                                RECIPROCAL_APPROX_FAST
    DVE: q,best = max((0 - interN) * rec)         custom IOU_MAX_ANT

(one relu suffices: the max accumulator is seeded at 0 and negative
phantom products only underestimate). Each core ships its 3 best-q
columns [128, 4] fp32; the host folds cores/slots/partitions. Slot det
capacities come from the input histogram at build time (SPMD: max over
cores per slot rank, padded to x16). fp16 det coords keep the final
loss within ~2e-5 of the fp32 reference.

NOTE device limits found the hard way: walrus rejects
scalar_tensor_tensor on Pool and tensor_tensor_reduce with op0=divide;
the fake_nrt runtime cannot execute InstReciprocal or
InstTensorTensorReduce at all (TimelineSim/CoreSim accept them). Stick
to tensor_scalar/tensor_tensor/custom-DVE ops for device runs.

Fallback ("dense" kernel): objects sharded 256/core on the free axis,
all dets on partitions, label mask applied explicitly - used when the
class layout doesn't fit (class > 128 objects, > 24 classes).
"""

from contextlib import ExitStack

import numpy as np

import concourse.bacc as bacc
import concourse.bass as bass
import concourse.mybir as mybir
import concourse.tile as tile
from concourse.bass_isa import ReduceOp
from concourse.dve_ops import RECIPROCAL_APPROX_FAST, RECIP_APPROX_FAST_CONSTS

_OPS_REGISTERED = {}


def _register_custom_ops():
    """Register the fused max-accum DVE op (extension point: dve_ops.OPS)."""
    if _OPS_REGISTERED:
        return _OPS_REGISTERED
    import concourse.dve_ops as dve_ops
    from concourse.dve_spec import (Spec, Src0, Src1, C0, C1, C2, Bin,
                                    AluOp, relu, minn, maxx, lower)
    from concourse.dve_uop import DveOpSpec

    def make(name, spec, subdim=False):
        if name in dve_ops._SUB_OPCODE_FOR_NAME:
            for op in dve_ops.OPS:
                if op.name == name:
                    return op
        row = dve_ops._CUSTOM_DVE_ROW_BASE + len(dve_ops.OPS)
        assert row < 0x20
        shas = {}
        from concourse.dve_spec import _has_src1
        for ver in ("v3", "v4"):
            uops = lower(spec, ver=ver)
            shas[ver] = DveOpSpec(name=name, opcode=row, uops=uops,
                                  rd1_en=_has_src1(spec)).sha(ver)
        op = dve_ops.DveOp(name, spec, subdim, shas)
        dve_ops.OPS.append(op)
        dve_ops.CUSTOM_DVE_SPECS[name] = spec
        dve_ops._SUB_OPCODE_FOR_NAME[name] = row
        return op

    def _wx_ref(in0, in1, s0, s1, imm2):
        import numpy as np
        return np.maximum(
            np.minimum(in0.astype(np.float32), s0)
            - np.maximum(in1.astype(np.float32), s1), 0.0)

    # wx = relu(min(d_hi, o_hi) - max(d_lo, o_lo))
    wx_op = make("IOU_WX_ANT", Spec(
        body=relu(minn(Src0, C0) - maxx(Src1, C1)),
        reference=_wx_ref,
    ))

    def _wxn_ref(in0, in1, s0, s1, imm2):
        import numpy as np
        return -np.maximum(
            np.minimum(in0.astype(np.float32), s0)
            - np.maximum(in1.astype(np.float32), s1), 0.0)

    # wxn = -relu(min(d_hi, o_hi) - max(d_lo, o_lo))
    wxn_op = make("IOU_WXN_ANT", Spec(
        body=-relu(minn(Src0, C0) - maxx(Src1, C1)),
        reference=_wxn_ref,
    ))

    def _ioumax_ref(in0, in1, s0, s1, imm2):
        import numpy as np
        b = ((s0 - in0.astype(np.float32)) * in1).astype(np.float32)
        b2 = b.reshape(b.shape[0], -1)
        seed = (np.asarray(s1, np.float32).reshape(-1, 1)
                if isinstance(s1, np.ndarray)
                else np.full((b2.shape[0], 1), s1, np.float32))
        return b, np.maximum(b2.max(axis=-1, keepdims=True), seed)

    # q = (C0 - in0) * in1 ; accum_out = max(q) over the free dim
    ioumax_op = make("IOU_MAX_ANT", Spec(
        body=(C0 - Src0) * Src1,
        accum=maxx,
        accum_init=C1,
        reference=_ioumax_ref,
    ))
    def _qmax_ref(in0, in1, s0, s1, imm2):
        import numpy as np
        nx = np.frombuffer(
            np.bitwise_not(in0.astype(np.float32).view(np.uint32)).tobytes(),
            dtype=np.float32).reshape(in0.shape)
        y0 = nx * np.float32(s0)
        r = y0 * (np.float32(s1) - in0.astype(np.float32) * y0)
        b = (in1.astype(np.float32) * r).astype(np.float32)
        b2 = b.reshape(b.shape[0], -1)
        return b, np.maximum(b2.max(axis=-1, keepdims=True),
                             np.float32(imm2)).astype(np.float32)

    # q = in1 * fastrecip(in0) (bitwise-not seed + one NR); accum = max.
    # Exactly 8 ALU stages — the 2-NR reciprocal does not fit fused.
    _nx = Bin(AluOp.BITWISE_NOT, Src0, Src0)
    _qy0 = _nx * C0
    qmax_op = make("IOU_QMAX_ANT", Spec(
        body=Src1 * (_qy0 * (C1 - Src0 * _qy0)),
        accum=maxx,
        accum_init=C2,
        reference=_qmax_ref,
    ))
    _OPS_REGISTERED.update(wx=wx_op, wxn=wxn_op, ioumax=ioumax_op,
                           qmax=qmax_op)
    return _OPS_REGISTERED
from concourse.bass_utils import run_bass_kernel_spmd

F32 = mybir.dt.float32
F16 = mybir.dt.float16
OP = mybir.AluOpType
AF = mybir.ActivationFunctionType
AX = mybir.AxisListType

N_CORES = 8
N_DET = 8732
N_OBJ = 2048
N_CLASSES = 21
OBJ_PER_CORE = N_OBJ // N_CORES  # 256
T_DET = 69                        # ceil(8732/128)
DET_PAD = 128 * T_DET             # 8832

S_SLOTS = 3
MAX_SLOTS = N_CORES * S_SLOTS

# det-row pad values: make Mx - Bx < 0 so relu kills padded columns
PAD_LO = 4.0     # pad for dx1/dy1 (lower corners)
PAD_HI = -4.0    # pad for dx2/dy2 (upper corners)
PAD_AD = 1.0     # pad det area (keeps s = ad + ao > 0)

# seed + one-NR reciprocal constants retuned for the single-NR fused qmax
# pipeline (the library pair is minimax-fit for two NR steps); centers the
# residual so the final loss bias is ~5e-6 instead of ~2e-4
QMAX_C0 = -0.2325
QMAX_C1 = 2.00125


def _build_fast(fds):
    """Class-bucketed kernel; fds = per-slot det capacities (len S_SLOTS).

    drow layout per slot s (fp16), at offset 5*sum(fds[:s]):
        [dx1 (f) | dx2 (f) | ad (f) | dy1 (f) | dy2 (f)]
    so the x+area rows [0:3f] and y rows [3f:5f] are each contiguous.
    objs layout (fp32): slot s cols 8s..8s+7 = [ox1,ox2,oy1,oy2,ao,0,0,0].
    Output qout [128, 4] fp32: col s = best q of slot s (col 3 unused).
    """
    ops = _register_custom_ops()
    nc = bacc.Bacc("TRN2", target_bir_lowering=False, debug=False,
                   num_devices=N_CORES)
    ftot = sum(fds)
    xoff = [2 * sum(fds[:s]) for s in range(S_SLOTS)]
    yoff = [2 * ftot + 2 * sum(fds[:s]) for s in range(S_SLOTS)]
    aoff = [sum(fds[:s]) for s in range(S_SLOTS)]

    drow_d = nc.dram_tensor("drow", [1, 4 * ftot], F16, kind="ExternalInput")
    # marow row0 = [ones(ftot) | ao_s(128) x3], row1 = [ad(ftot) | ones]:
    # per-slot PE matmul [ao_s,1]^T @ [[1],[ad]] = ao+ad -> PSUM f32
    marow_d = nc.dram_tensor("marow", [2, ftot + 128 * S_SLOTS], F16,
                             kind="ExternalInput")
    objs_d = nc.dram_tensor("objs", [128, S_SLOTS * 8], F32,
                            kind="ExternalInput")
    qout_d = nc.dram_tensor("qout", [128, 4], F32, kind="ExternalOutput")

    with tile.TileContext(nc) as tc, ExitStack() as ctx:
        cpool = ctx.enter_context(tc.tile_pool(name="const", bufs=1))
        wpool = ctx.enter_context(tc.tile_pool(name="work", bufs=2))
        ppool = ctx.enter_context(tc.tile_pool(name="psum", bufs=1,
                                               space="PSUM"))
        # object scalar columns ride the Pool queue (its descgen overlaps
        # the SP queue's first broadcast)
        objs = cpool.tile([128, S_SLOTS * 8], F32)
        nc.gpsimd.dma_start(objs[:], objs_d[:])

        qout = cpool.tile([128, 1, 4], F32)
        nc.vector.memset(qout[:], 0.0)

        # broadcast DMAs (SP queue, in landing order): slot0 split x/a/y for
        # the earliest possible DVE start; slots 1-2 as xa + y pairs.
        bt = {}
        def bcast(name, dram_lo, dram_hi, ftile):
            t = cpool.tile([128, ftile], F16, tag=name)
            src, _ = bass.broadcast_tensor_aps(
                drow_d[0:1, dram_lo:dram_hi], t[:])
            nc.sync.dma_start(t[:], src)
            bt[name] = t
            return t

        f0, f1, f2 = fds
        # landing order: slot0 x, slot0 y, slots 1+2 x (fused), det areas
        # (all slots), slot1 y, slot2 y
        bcast("b0x", xoff[0], xoff[0] + 2 * f0, 2 * f0)
        bcast("b0y", yoff[0], yoff[0] + 2 * f0, 2 * f0)
        bcast("b12x", xoff[1], xoff[1] + 2 * f1 + 2 * f2, 2 * f1 + 2 * f2)
        mar = cpool.tile([2, ftot + 128 * S_SLOTS], F16)
        nc.sync.dma_start(mar[:], marow_d[:])
        bcast("b1y", yoff[1], yoff[1] + 2 * f1, 2 * f1)
        bcast("b2y", yoff[2], yoff[2] + 2 * f2, 2 * f2)

        # s = ao + ad per slot on the (otherwise idle) PE, into PSUM
        sps = []
        for s in range(S_SLOTS):
            f = fds[s]
            aow = mar[0:2, ftot + 128 * s:ftot + 128 * (s + 1)]
            sp = ppool.tile([128, f], F32, tag=f"sp{s}")
            nc.tensor.matmul(sp[:], aow, mar[0:2, aoff[s]:aoff[s] + f],
                             start=True, stop=True)
            sps.append(sp)

        def rows(s):
            f = fds[s]
            if s == 0:
                dx1 = bt["b0x"][:, 0:f]
                dx2 = bt["b0x"][:, f:2 * f]
            elif f"b{s}x" in bt:
                dx1 = bt[f"b{s}x"][:, 0:f]
                dx2 = bt[f"b{s}x"][:, f:2 * f]
            else:
                xo = 2 * sum(fds[1:s])
                dx1 = bt["b12x"][:, xo:xo + f]
                dx2 = bt["b12x"][:, xo + f:xo + 2 * f]
            dy1 = bt[f"b{s}y"][:, 0:f]
            dy2 = bt[f"b{s}y"][:, f:2 * f]
            return dx1, dx2, dy1, dy2

        def ocol(s, k):
            return objs[:, 8 * s + k:8 * s + k + 1]

        # phase 1: x-side chains for every slot (DVE Bx -> Pool U -> Act Ur)
        urs = []
        for s in range(S_SLOTS):
            f = fds[s]
            dx1, dx2, dy1, dy2 = rows(s)
            wx = wpool.tile([128, f], F16, tag="wx", name=f"wx{s}")
            nc.vector._custom_dve(ops["wx"], out=wx[:], in0=dx2, in1=dx1,
                                  s0=ocol(s, 1), s1=ocol(s, 0))
            urs.append(wx)

        # phase 2: y-side + divide/max per slot on the DVE; ttrs last
        pend = []
        recs = []
        for s in range(S_SLOTS):
            f = fds[s]
            dx1, dx2, dy1, dy2 = rows(s)
            wy = wpool.tile([128, f], F16, tag="wy", name=f"wy{s}")
            nc.vector._custom_dve(ops["wx"], out=wy[:], in0=dy2, in1=dy1,
                                  s0=ocol(s, 3), s1=ocol(s, 2))
            inter = wpool.tile([128, f], F16, tag="inter", name=f"inter{s}")
            nc.vector.tensor_tensor(inter[:], urs[s][:], wy[:], OP.mult)
            pend.append(inter)
            if s == 0:
                scr = wpool.tile([128, f], F32, tag="scr", name=f"scr{s}")
                nc.vector._custom_dve(
                    ops["qmax"], out=scr[:], accum_out=qout[:, 0, s:s + 1],
                    in0=sps[s][:], in1=inter[:],
                    s0=QMAX_C0, s1=QMAX_C1, imm2=0.0)
        for s in range(1, S_SLOTS):
            f = fds[s]
            scr = wpool.tile([128, f], F32, tag="scr", name=f"scr{s}")
            nc.vector._custom_dve(
                ops["qmax"], out=scr[:], accum_out=qout[:, 0, s:s + 1],
                in0=sps[s][:], in1=pend[s][:],
                s0=QMAX_C0, s1=QMAX_C1, imm2=0.0)

        nc.sync.dma_start(qout_d[:], qout[:, 0, :])

    nc.compile()
    return nc


def _assign_classes(det_labels, labels):
    """Pick (class -> core, slot) or None if the layout doesn't fit.

    Returns (fds, assign) where assign[core][slot] = (cls, n_obj, n_det)
    or None for an empty slot; fds are slot det capacities (max over
    cores, padded to a multiple of 16).
    """
    if len(det_labels) == 0 or len(labels) == 0:
        return None
    if det_labels.min() < 0 or labels.min() < 0:
        return None
    ncls = int(max(N_CLASSES, det_labels.max() + 1, labels.max() + 1))
    dc = np.bincount(det_labels, minlength=ncls)
    oc = np.bincount(labels, minlength=ncls)
    active = np.where((dc > 0) & (oc > 0))[0]
    if len(active) > MAX_SLOTS or (oc[active] > 128).any():
        return None
    order = active[np.argsort(-dc[active], kind="stable")]
    # split the biggest classes' detections in half across the spare
    # slots (objects duplicated; the host maxes the piece q-columns)
    pieces = []
    nsplit = min(MAX_SLOTS - len(order), len(order))
    for r, cls in enumerate(order):
        nd = int(dc[cls])
        if r < nsplit and nd > 1:
            h = nd // 2
            pieces.append((int(cls), int(oc[cls]), h, 0))
            pieces.append((int(cls), int(oc[cls]), nd - h, h))
        else:
            pieces.append((int(cls), int(oc[cls]), nd, 0))
    pieces.sort(key=lambda p: -p[2])
    assign = [[None] * S_SLOTS for _ in range(N_CORES)]
    fds = [16] * S_SLOTS
    for r, p in enumerate(pieces):
        c, s = r % N_CORES, r // N_CORES
        assign[c][s] = p
        fds[s] = max(fds[s], p[2])
    fds = tuple(-(-f // 8) * 8 for f in fds)
    return fds, assign


def _prep_fast_inputs(det_boxes, det_labels, boxes, labels, fds, assign):
    det_boxes = det_boxes.astype(np.float32)
    boxes = boxes.astype(np.float32)
    ftot = sum(fds)
    xoff = [2 * sum(fds[:s]) for s in range(S_SLOTS)]
    yoff = [2 * ftot + 2 * sum(fds[:s]) for s in range(S_SLOTS)]
    aoff = [sum(fds[:s]) for s in range(S_SLOTS)]

    det_order = np.argsort(det_labels, kind="stable")
    obj_order = np.argsort(labels, kind="stable")
    ncls = int(max(N_CLASSES, det_labels.max() + 1, labels.max() + 1))
    dc = np.bincount(det_labels, minlength=ncls)
    oc = np.bincount(labels, minlength=ncls)
    det_off = np.concatenate([[0], np.cumsum(dc)])
    obj_off = np.concatenate([[0], np.cumsum(oc)])

    in_maps = []
    for c in range(N_CORES):
        drow = np.empty(4 * ftot, dtype=np.float16)
        marow = np.ones((2, ftot + 128 * S_SLOTS), dtype=np.float16)
        marow[1, :ftot] = PAD_AD
        objs = np.zeros((128, S_SLOTS * 8), dtype=np.float32)
        for s in range(S_SLOTS):
            f = fds[s]
            ox, oy = xoff[s], yoff[s]
            drow[ox + 0 * f:ox + 1 * f] = PAD_LO   # dx1
            drow[ox + 1 * f:ox + 2 * f] = PAD_HI   # dx2
            drow[oy + 0 * f:oy + 1 * f] = PAD_LO   # dy1
            drow[oy + 1 * f:oy + 2 * f] = PAD_HI   # dy2
            objs[:, 8 * s + 3] = 1.0   # benign pad box (0,0,0,1)
            objs[:, 8 * s + 4] = 1.0   # pad object area
            a = assign[c][s]
            if a is None:
                continue
            cls, no, nd, dlo = a
            dsel = det_order[det_off[cls] + dlo:det_off[cls] + dlo + nd]
            osel = obj_order[obj_off[cls]:obj_off[cls + 1]]
            d16 = det_boxes[dsel].astype(np.float16)   # (x1,y1,x2,y2)
            drow[ox + 0 * f:ox + 0 * f + nd] = d16[:, 0]   # dx1
            drow[ox + 1 * f:ox + 1 * f + nd] = d16[:, 2]   # dx2
            drow[oy + 0 * f:oy + 0 * f + nd] = d16[:, 1]   # dy1
            drow[oy + 1 * f:oy + 1 * f + nd] = d16[:, 3]   # dy2
            marow[1, aoff[s]:aoff[s] + nd] = (            # ad
                (d16[:, 2].astype(np.float32) - d16[:, 0]) *
                (d16[:, 3].astype(np.float32) - d16[:, 1])
            ).astype(np.float16)
            ob = boxes[osel]
            o16 = ob.astype(np.float16).astype(np.float32)
            objs[:no, 8 * s + 0] = o16[:, 0]
            objs[:no, 8 * s + 1] = o16[:, 2]
            objs[:no, 8 * s + 2] = o16[:, 1]
            objs[:no, 8 * s + 3] = o16[:, 3]
            objs[:no, 8 * s + 4] = ((o16[:, 2] - o16[:, 0]) *
                                    (o16[:, 3] - o16[:, 1]))
            marow[0, ftot + 128 * s:ftot + 128 * s + no] = (
                (o16[:, 2] - o16[:, 0]) * (o16[:, 3] - o16[:, 1])
            ).astype(np.float16)
        in_maps.append({"drow": drow.reshape(1, 4 * ftot), "marow": marow,
                        "objs": objs})
    return in_maps


def _fast_loss(results, assign):
    best = {}
    for c in range(N_CORES):
        q = results[c]["qout"]
        for s in range(S_SLOTS):
            a = assign[c][s]
            if a is None:
                continue
            cls, no, _, _ = a
            qs = np.clip(q[:no, s].astype(np.float64), 0.0, None)
            if cls in best:
                best[cls] = np.maximum(best[cls], qs)
            else:
                best[cls] = qs
    num = 0.0
    npos = 0
    for cls, qs in best.items():
        iou = qs / (1.0 - qs)
        num += float(np.sum(1.0 - iou))
        npos += len(qs)
    return np.asarray(np.float32(num / npos))


# ---------------------------------------------------------------------------
# dense fallback (any input)

def _build_dense():
    """Dense kernel: all dets (on partitions) x this core's objects (free)."""
    nc = bacc.Bacc("TRN2", target_bir_lowering=False, debug=False,
                   num_devices=N_CORES)
    F = OBJ_PER_CORE

    detp_d = nc.dram_tensor("detp", [128, 5, T_DET], F32, kind="ExternalInput")
    objr_d = nc.dram_tensor("objr", [5, F], F32, kind="ExternalInput")
    part_d = nc.dram_tensor("partial", [1, 2], F32, kind="ExternalOutput")

    with tile.TileContext(nc) as tc, ExitStack() as ctx:
        cpool = ctx.enter_context(tc.tile_pool(name="const", bufs=1))
        wpool = ctx.enter_context(tc.tile_pool(name="work", bufs=3))

        detp = cpool.tile([128, 5, T_DET], F32)
        nc.sync.dma_start(detp[:], detp_d[:])
        names = ["ox1", "oy1", "ox2", "oy2", "olab"]
        ob = {}
        for i, nm in enumerate(names):
            row = cpool.tile([1, F], F32, tag=f"r_{nm}")
            nc.sync.dma_start(row[:], objr_d[i:i + 1, :])
            t = cpool.tile([128, F], F32, tag=f"b_{nm}")
            nc.gpsimd.partition_broadcast(t[:], row[:], channels=128)
            ob[nm] = t

        aob = cpool.tile([128, F], F32)
        wob = wpool.tile([128, F], F32, tag="wob")
        nc.vector.tensor_tensor(wob[:], ob["ox2"][:], ob["ox1"][:], OP.subtract)
        hob = wpool.tile([128, F], F32, tag="hob")
        nc.vector.tensor_tensor(hob[:], ob["oy2"][:], ob["oy1"][:], OP.subtract)
        nc.vector.tensor_tensor(aob[:], wob[:], hob[:], OP.mult)

        ad = cpool.tile([128, T_DET], F32)
        wd = wpool.tile([128, T_DET], F32, tag="wd")
        nc.vector.tensor_tensor(wd[:], detp[:, 2, :], detp[:, 0, :], OP.subtract)
        hd = wpool.tile([128, T_DET], F32, tag="hd")
        nc.vector.tensor_tensor(hd[:], detp[:, 3, :], detp[:, 1, :], OP.subtract)
        nc.vector.tensor_tensor(ad[:], wd[:], hd[:], OP.mult)

        bmax = cpool.tile([128, F], F32)
        nc.vector.memset(bmax[:], 0.0)
        hm = cpool.tile([128, F], F32)
        nc.vector.memset(hm[:], 0.0)

        for t in range(T_DET):
            dx1 = detp[:, 0, t:t + 1]
            dy1 = detp[:, 1, t:t + 1]
            dx2 = detp[:, 2, t:t + 1]
            dy2 = detp[:, 3, t:t + 1]
            dlab = detp[:, 4, t:t + 1]
            adt = ad[:, t:t + 1]

            mnx = wpool.tile([128, F], F32, tag="mnx")
            nc.vector.tensor_scalar(mnx[:], ob["ox2"][:], dx2, None, op0=OP.min)
            mxx = wpool.tile([128, F], F32, tag="mxx")
            nc.vector.tensor_scalar(mxx[:], ob["ox1"][:], dx1, None, op0=OP.max)
            wx = wpool.tile([128, F], F32, tag="wx")
            nc.vector.tensor_tensor(wx[:], mnx[:], mxx[:], OP.subtract)
            wxr = wpool.tile([128, F], F32, tag="wxr")
            nc.vector.tensor_scalar(wxr[:], wx[:], 0.0, None, op0=OP.max)

            mny = wpool.tile([128, F], F32, tag="mny")
            nc.vector.tensor_scalar(mny[:], ob["oy2"][:], dy2, None, op0=OP.min)
            mxy = wpool.tile([128, F], F32, tag="mxy")
            nc.vector.tensor_scalar(mxy[:], ob["oy1"][:], dy1, None, op0=OP.max)
            wy = wpool.tile([128, F], F32, tag="wy")
            nc.vector.tensor_tensor(wy[:], mny[:], mxy[:], OP.subtract)
            wyr = wpool.tile([128, F], F32, tag="wyr")
            nc.vector.tensor_scalar(wyr[:], wy[:], 0.0, None, op0=OP.max)

            inter = wpool.tile([128, F], F32, tag="inter")
            nc.vector.tensor_tensor(inter[:], wxr[:], wyr[:], OP.mult)
            sab = wpool.tile([128, F], F32, tag="sab")
            nc.vector.tensor_scalar(sab[:], aob[:], adt, None, op0=OP.add)
            denom = wpool.tile([128, F], F32, tag="denom")
            nc.vector.tensor_tensor(denom[:], sab[:], inter[:], OP.subtract)
            rec = wpool.tile([128, F], F32, tag="rec")
            nc.vector.reciprocal(rec[:], denom[:])
            iou = wpool.tile([128, F], F32, tag="iou")
            nc.vector.tensor_tensor(iou[:], inter[:], rec[:], OP.mult)

            eq = wpool.tile([128, F], F32, tag="eq")
            nc.vector.tensor_scalar(eq[:], ob["olab"][:], dlab, None,
                                    op0=OP.is_equal)
            miou = wpool.tile([128, F], F32, tag="miou")
            nc.vector.tensor_tensor(miou[:], iou[:], eq[:], OP.mult)

            nc.vector.tensor_tensor(bmax[:], bmax[:], miou[:], OP.max)
            nc.vector.tensor_tensor(hm[:], hm[:], eq[:], OP.max)

        bred = cpool.tile([128, F], F32)
        nc.gpsimd.partition_all_reduce(bred[:], bmax[:], 128, ReduceOp.max)
        hred = cpool.tile([128, F], F32)
        nc.gpsimd.partition_all_reduce(hred[:], hm[:], 128, ReduceOp.max)

        c1 = wpool.tile([1, F], F32, tag="c1")
        nc.vector.tensor_scalar(c1[:], bred[0:1, :], -1.0, 1.0,
                                op0=OP.mult, op1=OP.add)
        c2 = wpool.tile([1, F], F32, tag="c2")
        nc.vector.tensor_tensor(c2[:], c1[:], hred[0:1, :], OP.mult)

        outt = wpool.tile([1, 2], F32, tag="outt")
        nc.vector.tensor_reduce(outt[:, 0:1], c2[:], AX.X, OP.add)
        nc.vector.tensor_reduce(outt[:, 1:2], hred[0:1, :], AX.X, OP.add)
        nc.sync.dma_start(part_d[:], outt[:])

    nc.compile()
    return nc


def _prep_dense_inputs(det_boxes, det_labels, boxes, labels):
    det = np.full((DET_PAD, 5), -5.0, dtype=np.float32)
    det[:N_DET, 0:4] = det_boxes.astype(np.float32)
    det[:N_DET, 4] = det_labels.astype(np.float32)
    det[N_DET:, 4] = -1.0
    detp = np.ascontiguousarray(
        det.reshape(T_DET, 128, 5).transpose(1, 2, 0))

    in_maps = []
    for c in range(N_CORES):
        sl = slice(c * OBJ_PER_CORE, (c + 1) * OBJ_PER_CORE)
        objr = np.empty((5, OBJ_PER_CORE), dtype=np.float32)
        objr[0:4, :] = boxes[sl].astype(np.float32).T
        objr[4, :] = labels[sl].astype(np.float32)
        in_maps.append({"detp": detp, "objr": objr})
    return in_maps


_CACHE = {}


def _get_dense():
    if "dense" not in _CACHE:
        _CACHE["dense"] = _build_dense()
    return _CACHE["dense"]


def _get_fast(fds):
    key = f"fast{fds}"
    if key not in _CACHE:
        _CACHE[key] = _build_fast(fds)
    return _CACHE[key]


def kernel(det_boxes, det_scores, det_labels, boxes, labels):
    det_boxes = np.asarray(det_boxes)
    det_labels = np.asarray(det_labels)
    boxes = np.asarray(boxes)
    labels = np.asarray(labels)

    plan = _assign_classes(det_labels, labels)
    if plan is not None:
        fds, assign = plan
        in_maps = _prep_fast_inputs(det_boxes, det_labels, boxes, labels,
                                    fds, assign)
        res = run_bass_kernel_spmd(_get_fast(fds), in_maps,
                                   list(range(N_CORES)))
        return _fast_loss(res.results, assign)

    in_maps = _prep_dense_inputs(det_boxes, det_labels, boxes, labels)
    res = run_bass_kernel_spmd(_get_dense(), in_maps, list(range(N_CORES)))
    tot = np.zeros(2, dtype=np.float32)
    for c in range(N_CORES):
        p = res.results[c]["partial"]
        tot += p.sum(axis=0, dtype=np.float32) if p.shape[0] > 1 else p[0]
    return np.asarray(np.float32(tot[0] / tot[1]))



# revision 3
# speedup vs baseline: 1.0090x; 1.0090x over previous
"""Trainium2 Bass kernel for nn_AttackLoss (nms_detection).

Computes, for O=2048 ground-truth boxes vs D=8732 detections:
    best[o]  = max IoU over same-label detections of object o
    loss     = sum(has_match * (1 - best)) / sum(has_match)

Fast path: classes are sorted by det count and sharded whole across the
8 cores x 3 slots (one class per slot; no label masking inside a slot).
Objects of the slot's class sit on SBUF partitions (corners as fp32
per-partition scalar columns), its detections along the free axis as
fp16 rows broadcast across partitions by stride-0 DMA reads straight
from DRAM.

Division-free max: q = inter/(ao+ad) = inter * G / ao with
G = ao/(ao+ad) = sigmoid(ln ao - ln ad). The idle Activation engine
computes G from a PE-broadcast ln(ad) row (sigmoid + relu share one
act table, so no table loads); the Pool engine multiplies wx*G for the
early slots (DVE tensor_tensor for the last); the final DVE op per
slot is a fused product+max-accumulate custom op. The host divides the
per-object max by ao and applies the monotone q -> iou = q/(1-q) map.
Per-slot engine schedule (f = slot det capacity):

    DVE:  wx = relu(min(dx2,ox2)-max(dx1,ox1))   custom (fp16 rows)
    DVE:  wy = same for y                        custom
    PE:   lnbc = ones x ln(ad) row -> PSUM       matmul broadcast
    Act:  G = sigmoid(lnao - lnbc)               activation (fp16 out)
    Pool: wxG = wx * G                           tensor_tensor
    DVE:  best = max-accum(wxG * wy)             custom PRODMAX

NOTE device limits found the hard way: walrus rejects
scalar_tensor_tensor on Pool and tensor_tensor_reduce with op0=divide;
the fake_nrt runtime cannot execute InstTensorTensorReduce at all
(TimelineSim/CoreSim accept it). Stick to tensor_scalar/tensor_tensor/
activation/custom-DVE ops for device runs.

Fallback ("dense" kernel): objects sharded 256/core on the free axis,
all dets on partitions, label mask applied explicitly - used when the
class layout doesn't fit (class > 128 objects, > 24 classes).
"""

from contextlib import ExitStack

import numpy as np

import concourse.bacc as bacc
import concourse.bass as bass
import concourse.mybir as mybir
import concourse.tile as tile
from concourse.bass_isa import ReduceOp

_OPS_REGISTERED = {}


def _register_custom_ops():
    """Register the fused DVE ops (extension point: dve_ops.OPS)."""
    if _OPS_REGISTERED:
        return _OPS_REGISTERED
    import concourse.dve_ops as dve_ops
    from concourse.dve_spec import (Spec, Src0, Src1, C0, C1,
                                    relu, minn, maxx)
    from concourse.dve_uop import DveOpSpec

    def make(name, spec, subdim=False):
        if name in dve_ops._SUB_OPCODE_FOR_NAME:
            for op in dve_ops.OPS:
                if op.name == name:
                    return op
        row = dve_ops._CUSTOM_DVE_ROW_BASE + len(dve_ops.OPS)
        assert row < 0x20
        shas = {}
        from concourse.dve_spec import _has_src1
        for ver in ("v3", "v4"):
            uops = lower(spec, ver=ver)
            shas[ver] = DveOpSpec(name=name, opcode=row, uops=uops,
                                  rd1_en=_has_src1(spec)).sha(ver)
        op = dve_ops.DveOp(name, spec, subdim, shas)
        dve_ops.OPS.append(op)
        dve_ops.CUSTOM_DVE_SPECS[name] = spec
        dve_ops._SUB_OPCODE_FOR_NAME[name] = row
        return op

    from concourse.dve_spec import lower

    def _wx_ref(in0, in1, s0, s1, imm2):
        import numpy as np
        return np.maximum(
            np.minimum(in0.astype(np.float32), s0)
            - np.maximum(in1.astype(np.float32), s1), 0.0)

    # wx = relu(min(d_hi, o_hi) - max(d_lo, o_lo))
    wx_op = make("IOU_WX_ANT", Spec(
        body=relu(minn(Src0, C0) - maxx(Src1, C1)),
        reference=_wx_ref,
    ))

    def _prodmax_ref(in0, in1, s0, s1, imm2):
        import numpy as np
        b = (in0.astype(np.float32) * in1.astype(np.float32)).astype(
            np.float32)
        b2 = b.reshape(b.shape[0], -1)
        seed = (np.asarray(s0, np.float32).reshape(-1, 1)
                if isinstance(s0, np.ndarray)
                else np.full((b2.shape[0], 1), s0, np.float32))
        return b, np.maximum(b2.max(axis=-1, keepdims=True), seed)

    # out = in0 * in1 ; accum_out = max(out) over the free dim, seed s0
    prodmax_op = make("IOU_PRODMAX_ANT", Spec(
        body=Src0 * Src1,
        accum=maxx,
        accum_init=C0,
        reference=_prodmax_ref,
    ))

    _OPS_REGISTERED.update(wx=wx_op, prodmax=prodmax_op)
    return _OPS_REGISTERED


from concourse.bass_utils import run_bass_kernel_spmd

F32 = mybir.dt.float32
F16 = mybir.dt.float16
OP = mybir.AluOpType
AF = mybir.ActivationFunctionType
AX = mybir.AxisListType

N_CORES = 8
N_DET = 8732
N_OBJ = 2048
N_CLASSES = 21
OBJ_PER_CORE = N_OBJ // N_CORES  # 256
T_DET = 69                        # ceil(8732/128)
DET_PAD = 128 * T_DET             # 8832

S_SLOTS = 3
MAX_SLOTS = N_CORES * S_SLOTS

# det-row pad values: make Mx - Bx < 0 so relu kills padded columns
PAD_LO = 4.0     # pad for dx1/dy1 (lower corners)
PAD_HI = -4.0    # pad for dx2/dy2 (upper corners)

# objs column roles (8 per slot)
OC_OX1, OC_OX2, OC_OY1, OC_OY2, OC_LNAO = 0, 1, 2, 3, 4


def _build_fast(fds):
    """Class-bucketed kernel; fds = per-slot det capacities (len S_SLOTS).

    drow layout (fp16): per slot s, x-rows [dx1 (f) | dx2 (f)] at
    xoff[s], y-rows [dy1 (f) | dy2 (f)] at yoff[s].
    lnrow (fp16, [1, ftot]): ln(det area) per slot at aoff[s].
    objs layout (fp32): slot s cols 8s.. = [ox1,ox2,oy1,oy2,lnao,0,0,0].
    Output qout [128, 4] fp32: col s = max(inter * ao/(ao+ad)) of slot s
    (col 3 unused); host divides by ao and maps q -> iou.
    """
    ops = _register_custom_ops()
    nc = bacc.Bacc("TRN2", target_bir_lowering=False, debug=False,
                   num_devices=N_CORES)
    ftot = sum(fds)
    xoff = [2 * sum(fds[:s]) for s in range(S_SLOTS)]
    yoff = [2 * ftot + 2 * sum(fds[:s]) for s in range(S_SLOTS)]
    aoff = [sum(fds[:s]) for s in range(S_SLOTS)]

    drow_d = nc.dram_tensor("drow", [1, 4 * ftot], F16, kind="ExternalInput")
    lnrow_d = nc.dram_tensor("lnrow", [1, ftot], F16, kind="ExternalInput")
    objs_d = nc.dram_tensor("objs", [128, S_SLOTS * 8], F32,
                            kind="ExternalInput")
    qout_d = nc.dram_tensor("qout", [128, 4], F32, kind="ExternalOutput")

    with tile.TileContext(nc) as tc, ExitStack() as ctx:
        cpool = ctx.enter_context(tc.tile_pool(name="const", bufs=1))
        wpool = ctx.enter_context(tc.tile_pool(name="work", bufs=2))
        ppool = ctx.enter_context(tc.tile_pool(name="psum", bufs=1,
                                               space="PSUM"))
        # object scalar columns + lnrow ride the Pool queue (SWDGE descgen
        # overlaps the SP queue's first broadcast)
        objs = cpool.tile([128, S_SLOTS * 8], F32)
        nc.gpsimd.dma_start(objs[:], objs_d[:])

        # ln(ad) source row + PE ones weights for the lnad broadcast
        lnr = cpool.tile([1, ftot], F16)
        nc.scalar.dma_start(lnr[:], lnrow_d[:])
        ones_w = cpool.tile([1, 128], F16)
        nc.gpsimd.memset(ones_w[:], 1.0)

        qout = cpool.tile([128, 1, 4], F32)
        nc.vector.memset(qout[:], 0.0)

        # broadcast DMAs (SP queue, in landing order): per-slot x then y.
        bt = {}
        def bcast(name, dram_lo, dram_hi, ftile):
            t = cpool.tile([128, ftile], F16, tag=name)
            src, _ = bass.broadcast_tensor_aps(
                drow_d[0:1, dram_lo:dram_hi], t[:])
            nc.sync.dma_start(t[:], src)
            bt[name] = t
            return t

        f0, f1, f2 = fds
        bcast("b0x", xoff[0], xoff[0] + 2 * f0, 2 * f0)
        bcast("b0y", yoff[0], yoff[0] + 2 * f0, 2 * f0)
        bcast("b1x", xoff[1], xoff[1] + 2 * f1, 2 * f1)
        bcast("b1y", yoff[1], yoff[1] + 2 * f1, 2 * f1)
        bcast("b2x", xoff[2], xoff[2] + 2 * f2, 2 * f2)
        bcast("b2y", yoff[2], yoff[2] + 2 * f2, 2 * f2)

        def rows(s):
            f = fds[s]
            bx, by = bt[f"b{s}x"], bt[f"b{s}y"]
            return (bx[:, 0:f], bx[:, f:2 * f],
                    by[:, 0:f], by[:, f:2 * f])

        def ocol(s, k):
            return objs[:, 8 * s + k:8 * s + k + 1]

        # PE: broadcast ln(ad) rows into PSUM (ones[1,128]^T @ lnr[1,f])
        lnbc = []
        for s in range(S_SLOTS):
            f = fds[s]
            ps = ppool.tile([128, f], F32, tag=f"lnbc{s}")
            nc.tensor.matmul(ps[:], ones_w[:], lnr[0:1, aoff[s]:aoff[s] + f],
                             start=True, stop=True)
            lnbc.append(ps)

        # Act: G = sigmoid(lnao - lnad) = ao/(ao+ad), fp16
        gts = []
        for s in range(S_SLOTS):
            f = fds[s]
            g = cpool.tile([128, f], F16, tag=f"g{s}")
            nc.scalar.activation(g[:], lnbc[s][:], AF.Sigmoid,
                                 bias=ocol(s, OC_LNAO), scale=-1.0)
            gts.append(g)

        # DVE: wx/wy custom chains per slot; Pool multiplies wx*G for
        # slots 0/1 (off the critical tail), DVE tt for slot 2.
        wxs, wys, wxgs = [], [], []
        for s in range(S_SLOTS):
            f = fds[s]
            dx1, dx2, dy1, dy2 = rows(s)
            wx = wpool.tile([128, f], F16, tag="wx", name=f"wx{s}")
            nc.vector._custom_dve(ops["wx"], out=wx[:], in0=dx2, in1=dx1,
                                  s0=ocol(s, OC_OX2), s1=ocol(s, OC_OX1))
            wxs.append(wx)
            wy = wpool.tile([128, f], F16, tag="wy", name=f"wy{s}")
            nc.vector._custom_dve(ops["wx"], out=wy[:], in0=dy2, in1=dy1,
                                  s0=ocol(s, OC_OY2), s1=ocol(s, OC_OY1))
            wys.append(wy)
            wxg = wpool.tile([128, f], F16, tag="wxg", name=f"wxg{s}")
            if s < 2:
                nc.gpsimd.tensor_tensor(wxg[:], wx[:], gts[s][:], OP.mult)
            else:
                nc.vector.tensor_tensor(wxg[:], wx[:], gts[s][:], OP.mult)
            wxgs.append(wxg)

        for s in range(S_SLOTS):
            f = fds[s]
            scr = wpool.tile([128, f], F16, tag="scr", name=f"scr{s}")
            nc.vector._custom_dve(
                ops["prodmax"], out=scr[:], accum_out=qout[:, 0, s:s + 1],
                in0=wxgs[s][:], in1=wys[s][:], s0=0.0)

        nc.sync.dma_start(qout_d[:], qout[:, 0, :])

    nc.compile()
    return nc


def _assign_classes(det_labels, labels):
    """Pick (class -> core, slot) or None if the layout doesn't fit.

    Returns (fds, assign) where assign[core][slot] = (cls, n_obj, n_det,
    det_lo) or None for an empty slot; fds are slot det capacities (max
    over cores, padded to a multiple of 8).
    """
    if len(det_labels) == 0 or len(labels) == 0:
        return None
    if det_labels.min() < 0 or labels.min() < 0:
        return None
    ncls = int(max(N_CLASSES, det_labels.max() + 1, labels.max() + 1))
    dc = np.bincount(det_labels, minlength=ncls)
    oc = np.bincount(labels, minlength=ncls)
    active = np.where((dc > 0) & (oc > 0))[0]
    if len(active) > MAX_SLOTS or (oc[active] > 128).any():
        return None
    order = active[np.argsort(-dc[active], kind="stable")]
    # split the biggest classes' detections in half across the spare
    # slots (objects duplicated; the host maxes the piece q-columns)
    pieces = []
    nsplit = min(MAX_SLOTS - len(order), len(order))
    for r, cls in enumerate(order):
        nd = int(dc[cls])
        if r < nsplit and nd > 1:
            h = nd // 2
            pieces.append((int(cls), int(oc[cls]), h, 0))
            pieces.append((int(cls), int(oc[cls]), nd - h, h))
        else:
            pieces.append((int(cls), int(oc[cls]), nd, 0))
    pieces.sort(key=lambda p: -p[2])
    assign = [[None] * S_SLOTS for _ in range(N_CORES)]
    fds = [16] * S_SLOTS
    for r, p in enumerate(pieces):
        c, s = r % N_CORES, r // N_CORES
        assign[c][s] = p
        fds[s] = max(fds[s], p[2])
    fds = tuple(-(-f // 8) * 8 for f in fds)
    return fds, assign


def _prep_fast_inputs(det_boxes, det_labels, boxes, labels, fds, assign):
    det_boxes = det_boxes.astype(np.float32)
    boxes = boxes.astype(np.float32)
    ftot = sum(fds)
    xoff = [2 * sum(fds[:s]) for s in range(S_SLOTS)]
    yoff = [2 * ftot + 2 * sum(fds[:s]) for s in range(S_SLOTS)]
    aoff = [sum(fds[:s]) for s in range(S_SLOTS)]

    det_order = np.argsort(det_labels, kind="stable")
    obj_order = np.argsort(labels, kind="stable")
    ncls = int(max(N_CLASSES, det_labels.max() + 1, labels.max() + 1))
    dc = np.bincount(det_labels, minlength=ncls)
    oc = np.bincount(labels, minlength=ncls)
    det_off = np.concatenate([[0], np.cumsum(dc)])
    obj_off = np.concatenate([[0], np.cumsum(oc)])

    in_maps = []
    aos = []   # per (core, slot): fp64 object areas for the host division
    for c in range(N_CORES):
        drow = np.empty(4 * ftot, dtype=np.float16)
        lnrow = np.zeros(ftot, dtype=np.float16)
        objs = np.zeros((128, S_SLOTS * 8), dtype=np.float32)
        ao_cs = [None] * S_SLOTS
        for s in range(S_SLOTS):
            f = fds[s]
            ox, oy = xoff[s], yoff[s]
            drow[ox + 0 * f:ox + 1 * f] = PAD_LO   # dx1
            drow[ox + 1 * f:ox + 2 * f] = PAD_HI   # dx2
            drow[oy + 0 * f:oy + 1 * f] = PAD_LO   # dy1
            drow[oy + 1 * f:oy + 2 * f] = PAD_HI   # dy2
            objs[:, 8 * s + OC_OY2] = 1.0          # benign pad box
            a = assign[c][s]
            if a is None:
                continue
            cls, no, nd, dlo = a
            dsel = det_order[det_off[cls] + dlo:det_off[cls] + dlo + nd]
            osel = obj_order[obj_off[cls]:obj_off[cls + 1]]
            d16 = det_boxes[dsel].astype(np.float16)   # (x1,y1,x2,y2)
            drow[ox + 0 * f:ox + 0 * f + nd] = d16[:, 0]   # dx1
            drow[ox + 1 * f:ox + 1 * f + nd] = d16[:, 2]   # dx2
            drow[oy + 0 * f:oy + 0 * f + nd] = d16[:, 1]   # dy1
            drow[oy + 1 * f:oy + 1 * f + nd] = d16[:, 3]   # dy2
            ad = ((d16[:, 2].astype(np.float64) - d16[:, 0]) *
                  (d16[:, 3].astype(np.float64) - d16[:, 1]))
            lnrow[aoff[s]:aoff[s] + nd] = np.log(
                np.maximum(ad, 1e-12)).astype(np.float16)
            ob = boxes[osel]
            o16 = ob.astype(np.float16).astype(np.float32)
            objs[:no, 8 * s + OC_OX1] = o16[:, 0]
            objs[:no, 8 * s + OC_OX2] = o16[:, 2]
            objs[:no, 8 * s + OC_OY1] = o16[:, 1]
            objs[:no, 8 * s + OC_OY2] = o16[:, 3]
            ao = ((o16[:, 2] - o16[:, 0]) * (o16[:, 3] - o16[:, 1]))
            objs[:no, 8 * s + OC_LNAO] = np.log(
                np.maximum(ao.astype(np.float64), 1e-12)).astype(np.float32)
            ao_cs[s] = ao.astype(np.float64)
        in_maps.append({"drow": drow.reshape(1, 4 * ftot),
                        "lnrow": lnrow.reshape(1, ftot), "objs": objs})
        aos.append(ao_cs)
    return in_maps, aos


def _fast_loss(results, assign, aos):
    best = {}
    for c in range(N_CORES):
        q = results[c]["qout"]
        for s in range(S_SLOTS):
            a = assign[c][s]
            if a is None:
                continue
            cls, no, _, _ = a
            ao = aos[c][s]
            qs = np.clip(q[:no, s].astype(np.float64) / ao, 0.0, None)
            if cls in best:
                best[cls] = np.maximum(best[cls], qs)
            else:
                best[cls] = qs
    num = 0.0
    npos = 0
    for cls, qs in best.items():
        iou = qs / (1.0 - qs)
        num += float(np.sum(1.0 - iou))
        npos += len(qs)
    return np.asarray(np.float32(num / npos))


# ---------------------------------------------------------------------------
# dense fallback (any input)

def _build_dense():
    """Dense kernel: all dets (on partitions) x this core's objects (free)."""
    nc = bacc.Bacc("TRN2", target_bir_lowering=False, debug=False,
                   num_devices=N_CORES)
    F = OBJ_PER_CORE

    detp_d = nc.dram_tensor("detp", [128, 5, T_DET], F32, kind="ExternalInput")
    objr_d = nc.dram_tensor("objr", [5, F], F32, kind="ExternalInput")
    part_d = nc.dram_tensor("partial", [1, 2], F32, kind="ExternalOutput")

    with tile.TileContext(nc) as tc, ExitStack() as ctx:
        cpool = ctx.enter_context(tc.tile_pool(name="const", bufs=1))
        wpool = ctx.enter_context(tc.tile_pool(name="work", bufs=3))

        detp = cpool.tile([128, 5, T_DET], F32)
        nc.sync.dma_start(detp[:], detp_d[:])
        names = ["ox1", "oy1", "ox2", "oy2", "olab"]
        ob = {}
        for i, nm in enumerate(names):
            row = cpool.tile([1, F], F32, tag=f"r_{nm}")
            nc.sync.dma_start(row[:], objr_d[i:i + 1, :])
            t = cpool.tile([128, F], F32, tag=f"b_{nm}")
            nc.gpsimd.partition_broadcast(t[:], row[:], channels=128)
            ob[nm] = t

        aob = cpool.tile([128, F], F32)
        wob = wpool.tile([128, F], F32, tag="wob")
        nc.vector.tensor_tensor(wob[:], ob["ox2"][:], ob["ox1"][:], OP.subtract)
        hob = wpool.tile([128, F], F32, tag="hob")
        nc.vector.tensor_tensor(hob[:], ob["oy2"][:], ob["oy1"][:], OP.subtract)
        nc.vector.tensor_tensor(aob[:], wob[:], hob[:], OP.mult)

        ad = cpool.tile([128, T_DET], F32)
        wd = wpool.tile([128, T_DET], F32, tag="wd")
        nc.vector.tensor_tensor(wd[:], detp[:, 2, :], detp[:, 0, :], OP.subtract)
        hd = wpool.tile([128, T_DET], F32, tag="hd")
        nc.vector.tensor_tensor(hd[:], detp[:, 3, :], detp[:, 1, :], OP.subtract)
        nc.vector.tensor_tensor(ad[:], wd[:], hd[:], OP.mult)

        bmax = cpool.tile([128, F], F32)
        nc.vector.memset(bmax[:], 0.0)
        hm = cpool.tile([128, F], F32)
        nc.vector.memset(hm[:], 0.0)

        for t in range(T_DET):
            dx1 = detp[:, 0, t:t + 1]
            dy1 = detp[:, 1, t:t + 1]
            dx2 = detp[:, 2, t:t + 1]
            dy2 = detp[:, 3, t:t + 1]
            dlab = detp[:, 4, t:t + 1]
            adt = ad[:, t:t + 1]

            mnx = wpool.tile([128, F], F32, tag="mnx")
            nc.vector.tensor_scalar(mnx[:], ob["ox2"][:], dx2, None, op0=OP.min)
            mxx = wpool.tile([128, F], F32, tag="mxx")
            nc.vector.tensor_scalar(mxx[:], ob["ox1"][:], dx1, None, op0=OP.max)
            wx = wpool.tile([128, F], F32, tag="wx")
            nc.vector.tensor_tensor(wx[:], mnx[:], mxx[:], OP.subtract)
            wxr = wpool.tile([128, F], F32, tag="wxr")
            nc.vector.tensor_scalar(wxr[:], wx[:], 0.0, None, op0=OP.max)

            mny = wpool.tile([128, F], F32, tag="mny")
            nc.vector.tensor_scalar(mny[:], ob["oy2"][:], dy2, None, op0=OP.min)
            mxy = wpool.tile([128, F], F32, tag="mxy")
            nc.vector.tensor_scalar(mxy[:], ob["oy1"][:], dy1, None, op0=OP.max)
            wy = wpool.tile([128, F], F32, tag="wy")
            nc.vector.tensor_tensor(wy[:], mny[:], mxy[:], OP.subtract)
            wyr = wpool.tile([128, F], F32, tag="wyr")
            nc.vector.tensor_scalar(wyr[:], wy[:], 0.0, None, op0=OP.max)

            inter = wpool.tile([128, F], F32, tag="inter")
            nc.vector.tensor_tensor(inter[:], wxr[:], wyr[:], OP.mult)
            sab = wpool.tile([128, F], F32, tag="sab")
            nc.vector.tensor_scalar(sab[:], aob[:], adt, None, op0=OP.add)
            denom = wpool.tile([128, F], F32, tag="denom")
            nc.vector.tensor_tensor(denom[:], sab[:], inter[:], OP.subtract)
            rec = wpool.tile([128, F], F32, tag="rec")
            nc.vector.reciprocal(rec[:], denom[:])
            iou = wpool.tile([128, F], F32, tag="iou")
            nc.vector.tensor_tensor(iou[:], inter[:], rec[:], OP.mult)

            eq = wpool.tile([128, F], F32, tag="eq")
            nc.vector.tensor_scalar(eq[:], ob["olab"][:], dlab, None,
                                    op0=OP.is_equal)
            miou = wpool.tile([128, F], F32, tag="miou")
            nc.vector.tensor_tensor(miou[:], iou[:], eq[:], OP.mult)

            nc.vector.tensor_tensor(bmax[:], bmax[:], miou[:], OP.max)
            nc.vector.tensor_tensor(hm[:], hm[:], eq[:], OP.max)

        bred = cpool.tile([128, F], F32)
        nc.gpsimd.partition_all_reduce(bred[:], bmax[:], 128, ReduceOp.max)
        hred = cpool.tile([128, F], F32)
        nc.gpsimd.partition_all_reduce(hred[:], hm[:], 128, ReduceOp.max)

        c1 = wpool.tile([1, F], F32, tag="c1")
        nc.vector.tensor_scalar(c1[:], bred[0:1, :], -1.0, 1.0,
                                op0=OP.mult, op1=OP.add)
        c2 = wpool.tile([1, F], F32, tag="c2")
        nc.vector.tensor_tensor(c2[:], c1[:], hred[0:1, :], OP.mult)

        outt = wpool.tile([1, 2], F32, tag="outt")
        nc.vector.tensor_reduce(outt[:, 0:1], c2[:], AX.X, OP.add)
        nc.vector.tensor_reduce(outt[:, 1:2], hred[0:1, :], AX.X, OP.add)
        nc.sync.dma_start(part_d[:], outt[:])

    nc.compile()
    return nc


def _prep_dense_inputs(det_boxes, det_labels, boxes, labels):
    det = np.full((DET_PAD, 5), -5.0, dtype=np.float32)
    det[:N_DET, 0:4] = det_boxes.astype(np.float32)
    det[:N_DET, 4] = det_labels.astype(np.float32)
    det[N_DET:, 4] = -1.0
    detp = np.ascontiguousarray(
        det.reshape(T_DET, 128, 5).transpose(1, 2, 0))

    in_maps = []
    for c in range(N_CORES):
        sl = slice(c * OBJ_PER_CORE, (c + 1) * OBJ_PER_CORE)
        objr = np.empty((5, OBJ_PER_CORE), dtype=np.float32)
        objr[0:4, :] = boxes[sl].astype(np.float32).T
        objr[4, :] = labels[sl].astype(np.float32)
        in_maps.append({"detp": detp, "objr": objr})
    return in_maps


_CACHE = {}


def _get_dense():
    if "dense" not in _CACHE:
        _CACHE["dense"] = _build_dense()
    return _CACHE["dense"]


def _get_fast(fds):
    key = f"fast{fds}"
    if key not in _CACHE:
        _CACHE[key] = _build_fast(fds)
    return _CACHE[key]


def kernel(det_boxes, det_scores, det_labels, boxes, labels):
    det_boxes = np.asarray(det_boxes)
    det_labels = np.asarray(det_labels)
    boxes = np.asarray(boxes)
    labels = np.asarray(labels)

    plan = _assign_classes(det_labels, labels)
    if plan is not None:
        fds, assign = plan
        in_maps, aos = _prep_fast_inputs(det_boxes, det_labels, boxes,
                                         labels, fds, assign)
        res = run_bass_kernel_spmd(_get_fast(fds), in_maps,
                                   list(range(N_CORES)))
        return _fast_loss(res.results, assign, aos)

    in_maps = _prep_dense_inputs(det_boxes, det_labels, boxes, labels)
    res = run_bass_kernel_spmd(_get_dense(), in_maps, list(range(N_CORES)))
    tot = np.zeros(2, dtype=np.float32)
    for c in range(N_CORES):
        p = res.results[c]["partial"]
        tot += p.sum(axis=0, dtype=np.float32) if p.shape[0] > 1 else p[0]
    return np.asarray(np.float32(tot[0] / tot[1]))


# revision 8
# speedup vs baseline: 1.0575x; 1.0481x over previous
"""Trainium2 Bass kernel for nn_AttackLoss (nms_detection).

Computes, for O=2048 ground-truth boxes vs D=8732 detections:
    best[o]  = max IoU over same-label detections of object o
    loss     = sum(has_match * (1 - best)) / sum(has_match)

Fast path: classes are sorted by det count and sharded whole across the
8 cores x 3 slots (one class per slot; no label masking inside a slot).
Objects of the slot's class sit on SBUF partitions (corners as fp32
per-partition scalar columns), its detections along the free axis as
fp16 rows broadcast across partitions by stride-0 DMA reads straight
from DRAM. The IoU max uses the monotone identity iou = q/(1-q) with
q = inter/(ao+ad): max_d iou = g(max_d q), so no per-pair union-minus-
intersection is needed and the host does the per-object q->iou map.
Per-slot passes (fp16 coords feed the DVE's 2x/4x fast modes; the
division tail stays fp32 for the bit-trick reciprocal):

    DVE: Bx = max(dx1, ox1); Mx = min(dx2, ox2)   tensor_scalar (4x)
    DVE: U = Mx - Bx                              tensor_tensor (2x)
    DVE: Urn = -max(U, 0)                         tensor_scalar (4x)
    DVE: By, My, V = My - By                      same for y
    DVE: interN = Urn * V                         tensor_tensor (2x)
    DVE: s = ad + ao  (fp32 out)                  tensor_scalar
    DVE: rec ~ 1/s                                RECIPROCAL_APPROX_FAST
    DVE: q,best = max((0 - interN) * rec)         custom IOU_MAX_ANT

(one relu suffices: the max accumulator is seeded at 0 and negative
phantom products only underestimate). Each core ships its 3 best-q
columns [128, 4] fp32; the host folds cores/slots/partitions. Slot det
capacities come from the input histogram at build time (SPMD: max over
cores per slot rank, padded to x16). fp16 det coords keep the final
loss within ~2e-5 of the fp32 reference.

NOTE device limits found the hard way: walrus rejects
scalar_tensor_tensor on Pool and tensor_tensor_reduce with op0=divide;
the fake_nrt runtime cannot execute InstReciprocal or
InstTensorTensorReduce at all (TimelineSim/CoreSim accept them). Stick
to tensor_scalar/tensor_tensor/custom-DVE ops for device runs.

Fallback ("dense" kernel): objects sharded 256/core on the free axis,
all dets on partitions, label mask applied explicitly - used when the
class layout doesn't fit (class > 128 objects, > 24 classes).
"""

from contextlib import ExitStack

import numpy as np

import concourse.bacc as bacc
import concourse.bass as bass
import concourse.mybir as mybir
import concourse.tile as tile
from concourse.bass_isa import ReduceOp
from concourse.dve_ops import RECIPROCAL_APPROX_FAST, RECIP_APPROX_FAST_CONSTS

_OPS_REGISTERED = {}


def _register_custom_ops():
    """Register the fused max-accum DVE op (extension point: dve_ops.OPS)."""
    if _OPS_REGISTERED:
        return _OPS_REGISTERED
    import concourse.dve_ops as dve_ops
    from concourse.dve_spec import (Spec, Src0, Src1, C0, C1, C2, Bin,
                                    AluOp, relu, minn, maxx, lower)
    from concourse.dve_uop import DveOpSpec

    def make(name, spec, subdim=False):
        if name in dve_ops._SUB_OPCODE_FOR_NAME:
            for op in dve_ops.OPS:
                if op.name == name:
                    return op
        row = dve_ops._CUSTOM_DVE_ROW_BASE + len(dve_ops.OPS)
        assert row < 0x20
        shas = {}
        from concourse.dve_spec import _has_src1
        for ver in ("v3", "v4"):
            uops = lower(spec, ver=ver)
            shas[ver] = DveOpSpec(name=name, opcode=row, uops=uops,
                                  rd1_en=_has_src1(spec)).sha(ver)
        op = dve_ops.DveOp(name, spec, subdim, shas)
        dve_ops.OPS.append(op)
        dve_ops.CUSTOM_DVE_SPECS[name] = spec
        dve_ops._SUB_OPCODE_FOR_NAME[name] = row
        return op

    def _wx_ref(in0, in1, s0, s1, imm2):
        import numpy as np
        return np.maximum(
            np.minimum(in0.astype(np.float32), s0)
            - np.maximum(in1.astype(np.float32), s1), 0.0)

    # wx = relu(min(d_hi, o_hi) - max(d_lo, o_lo))
    wx_op = make("IOU_WX_ANT", Spec(
        body=relu(minn(Src0, C0) - maxx(Src1, C1)),
        reference=_wx_ref,
    ))

    def _wxn_ref(in0, in1, s0, s1, imm2):
        import numpy as np
        return -np.maximum(
            np.minimum(in0.astype(np.float32), s0)
            - np.maximum(in1.astype(np.float32), s1), 0.0)

    # wxn = -relu(min(d_hi, o_hi) - max(d_lo, o_lo))
    wxn_op = make("IOU_WXN_ANT", Spec(
        body=-relu(minn(Src0, C0) - maxx(Src1, C1)),
        reference=_wxn_ref,
    ))

    def _ioumax_ref(in0, in1, s0, s1, imm2):
        import numpy as np
        b = ((s0 - in0.astype(np.float32)) * in1).astype(np.float32)
        b2 = b.reshape(b.shape[0], -1)
        seed = (np.asarray(s1, np.float32).reshape(-1, 1)
                if isinstance(s1, np.ndarray)
                else np.full((b2.shape[0], 1), s1, np.float32))
        return b, np.maximum(b2.max(axis=-1, keepdims=True), seed)

    # q = (C0 - in0) * in1 ; accum_out = max(q) over the free dim
    ioumax_op = make("IOU_MAX_ANT", Spec(
        body=(C0 - Src0) * Src1,
        accum=maxx,
        accum_init=C1,
        reference=_ioumax_ref,
    ))
    def _qmax_ref(in0, in1, s0, s1, imm2):
        import numpy as np
        nx = np.frombuffer(
            np.bitwise_not(in0.astype(np.float32).view(np.uint32)).tobytes(),
            dtype=np.float32).reshape(in0.shape)
        y0 = nx * np.float32(s0)
        r = y0 * (np.float32(s1) - in0.astype(np.float32) * y0)
        b = (in1.astype(np.float32) * r).astype(np.float32)
        b2 = b.reshape(b.shape[0], -1)
        return b, np.maximum(b2.max(axis=-1, keepdims=True),
                             np.float32(imm2)).astype(np.float32)

    # q = in1 * fastrecip(in0) (bitwise-not seed + one NR); accum = max.
    # Exactly 8 ALU stages — the 2-NR reciprocal does not fit fused.
    _nx = Bin(AluOp.BITWISE_NOT, Src0, Src0)
    _qy0 = _nx * C0
    qmax_op = make("IOU_QMAX_ANT", Spec(
        body=Src1 * (_qy0 * (C1 - Src0 * _qy0)),
        accum=maxx,
        accum_init=C2,
        reference=_qmax_ref,
    ))
    _OPS_REGISTERED.update(wx=wx_op, wxn=wxn_op, ioumax=ioumax_op,
                           qmax=qmax_op)
    return _OPS_REGISTERED
from concourse.bass_utils import run_bass_kernel_spmd

F32 = mybir.dt.float32
F16 = mybir.dt.float16
OP = mybir.AluOpType
AF = mybir.ActivationFunctionType
AX = mybir.AxisListType

N_CORES = 8
N_DET = 8732
N_OBJ = 2048
N_CLASSES = 21
OBJ_PER_CORE = N_OBJ // N_CORES  # 256
T_DET = 69                        # ceil(8732/128)
DET_PAD = 128 * T_DET             # 8832

S_SLOTS = 3
MAX_SLOTS = N_CORES * S_SLOTS

# det-row pad values: make Mx - Bx < 0 so relu kills padded columns
PAD_LO = 4.0     # pad for dx1/dy1 (lower corners)
PAD_HI = -4.0    # pad for dx2/dy2 (upper corners)
PAD_AD = 1.0     # pad det area (keeps s = ad + ao > 0)

# seed + one-NR reciprocal constants retuned for the single-NR fused qmax
# pipeline (the library pair is minimax-fit for two NR steps); centers the
# residual so the final loss bias is ~5e-6 instead of ~2e-4
QMAX_C0 = -0.2325
QMAX_C1 = 2.00125


def _build_fast(fds):
    """Class-bucketed kernel; fds = per-slot det capacities (len S_SLOTS).

    drow layout per slot s (fp16), at offset 5*sum(fds[:s]):
        [dx1 (f) | dx2 (f) | ad (f) | dy1 (f) | dy2 (f)]
    so the x+area rows [0:3f] and y rows [3f:5f] are each contiguous.
    objs layout (fp32): slot s cols 8s..8s+7 = [ox1,ox2,oy1,oy2,ao,0,0,0].
    Output qout [128, 4] fp32: col s = best q of slot s (col 3 unused).
    """
    ops = _register_custom_ops()
    nc = bacc.Bacc("TRN2", target_bir_lowering=False, debug=False,
                   num_devices=N_CORES)
    ftot = sum(fds)
    xoff = [2 * sum(fds[:s]) for s in range(S_SLOTS)]
    yoff = [2 * ftot + 2 * sum(fds[:s]) for s in range(S_SLOTS)]
    aoff = [sum(fds[:s]) for s in range(S_SLOTS)]

    drow_d = nc.dram_tensor("drow", [1, 4 * ftot], F16, kind="ExternalInput")
    # marow row0 = [ones(ftot) | ao_s(128) x3], row1 = [ad(ftot) | ones]:
    # per-slot PE matmul [ao_s,1]^T @ [[1],[ad]] = ao+ad -> PSUM f32
    marow_d = nc.dram_tensor("marow", [2, ftot + 128 * S_SLOTS], F16,
                             kind="ExternalInput")
    objs_d = nc.dram_tensor("objs", [128, S_SLOTS * 8], F32,
                            kind="ExternalInput")
    qout_d = nc.dram_tensor("qout", [128, 4], F32, kind="ExternalOutput")

    with tile.TileContext(nc) as tc, ExitStack() as ctx:
        cpool = ctx.enter_context(tc.tile_pool(name="const", bufs=1))
        wpool = ctx.enter_context(tc.tile_pool(name="work", bufs=2))
        ppool = ctx.enter_context(tc.tile_pool(name="psum", bufs=1,
                                               space="PSUM"))
        # object scalar columns + marow ride the Pool queue (SWDGE descgen
        # overlaps the SP queue's broadcasts and costs no HWDGE slot)
        objs = cpool.tile([128, S_SLOTS * 8], F32)
        nc.gpsimd.dma_start(objs[:], objs_d[:])
        mar = cpool.tile([2, ftot + 128 * S_SLOTS], F16)
        nc.gpsimd.dma_start(mar[:], marow_d[:])

        qout = cpool.tile([128, 1, 4], F32)
        nc.vector.memset(qout[:], 0.0)

        # broadcast DMAs (SP queue, in landing order): per-slot x then y,
        # so each slot's DVE chain unblocks as early as possible.
        bt = {}
        def bcast(name, dram_lo, dram_hi, ftile):
            t = cpool.tile([128, ftile], F16, tag=name)
            src, _ = bass.broadcast_tensor_aps(
                drow_d[0:1, dram_lo:dram_hi], t[:])
            nc.sync.dma_start(t[:], src)
            bt[name] = t
            return t

        f0, f1, f2 = fds
        bcast("b0x", xoff[0], xoff[0] + 2 * f0, 2 * f0)
        bcast("b0y", yoff[0], yoff[0] + 2 * f0, 2 * f0)
        bcast("b1x", xoff[1], xoff[1] + 2 * f1, 2 * f1)
        bcast("b1y", yoff[1], yoff[1] + 2 * f1, 2 * f1)
        bcast("b2x", xoff[2], xoff[2] + 2 * f2, 2 * f2)
        bcast("b2y", yoff[2], yoff[2] + 2 * f2, 2 * f2)

        # s = ao + ad per slot on the (otherwise idle) PE, into PSUM
        sps = []
        for s in range(S_SLOTS):
            f = fds[s]
            aow = mar[0:2, ftot + 128 * s:ftot + 128 * (s + 1)]
            sp = ppool.tile([128, f], F32, tag=f"sp{s}")
            nc.tensor.matmul(sp[:], aow, mar[0:2, aoff[s]:aoff[s] + f],
                             start=True, stop=True)
            sps.append(sp)

        def rows(s):
            f = fds[s]
            bx, by = bt[f"b{s}x"], bt[f"b{s}y"]
            return (bx[:, 0:f], bx[:, f:2 * f],
                    by[:, 0:f], by[:, f:2 * f])

        def ocol(s, k):
            return objs[:, 8 * s + k:8 * s + k + 1]

        # DVE stream: wx0,wy0,wx1,wy1,qmax0,wx2,wy2,inter2,qmax1,qmax2.
        # inter for slots 0/1 runs on the (otherwise idle) Pool engine in
        # the shadow of the DVE customs; slot 2's inter stays on the DVE
        # so the post-b2y tail chain has no cross-engine hop.
        wxs, wys, inters = [None] * S_SLOTS, [None] * S_SLOTS, [None] * S_SLOTS

        def emit_wx(s):
            f = fds[s]
            dx1, dx2, _, _ = rows(s)
            wx = wpool.tile([128, f], F16, tag="wx", name=f"wx{s}")
            nc.vector._custom_dve(ops["wx"], out=wx[:], in0=dx2, in1=dx1,
                                  s0=ocol(s, 1), s1=ocol(s, 0))
            wxs[s] = wx

        def emit_wy(s):
            f = fds[s]
            _, _, dy1, dy2 = rows(s)
            wy = wpool.tile([128, f], F16, tag="wy", name=f"wy{s}")
            nc.vector._custom_dve(ops["wx"], out=wy[:], in0=dy2, in1=dy1,
                                  s0=ocol(s, 3), s1=ocol(s, 2))
            wys[s] = wy

        def emit_inter(s, engine):
            f = fds[s]
            inter = wpool.tile([128, f], F16, tag="inter", name=f"inter{s}")
            engine.tensor_tensor(inter[:], wxs[s][:], wys[s][:], OP.mult)
            inters[s] = inter

        def emit_qmax(s):
            f = fds[s]
            scr = wpool.tile([128, f], F32, tag="scr", name=f"scr{s}")
            nc.vector._custom_dve(
                ops["qmax"], out=scr[:], accum_out=qout[:, 0, s:s + 1],
                in0=sps[s][:], in1=inters[s][:],
                s0=QMAX_C0, s1=QMAX_C1, imm2=0.0)

        emit_wx(0)
        emit_wy(0)
        emit_inter(0, nc.gpsimd)
        emit_wx(1)
        emit_wy(1)
        emit_inter(1, nc.gpsimd)
        emit_qmax(0)
        emit_wx(2)
        emit_wy(2)
        emit_inter(2, nc.vector)
        emit_qmax(1)
        emit_qmax(2)

        nc.sync.dma_start(qout_d[:], qout[:, 0, :])

    nc.compile()
    return nc


def _assign_classes(det_labels, labels):
    """Pick (class -> core, slot) or None if the layout doesn't fit.

    Returns (fds, assign) where assign[core][slot] = (cls, n_obj, n_det)
    or None for an empty slot; fds are slot det capacities (max over
    cores, padded to a multiple of 16).
    """
    if len(det_labels) == 0 or len(labels) == 0:
        return None
    if det_labels.min() < 0 or labels.min() < 0:
        return None
    ncls = int(max(N_CLASSES, det_labels.max() + 1, labels.max() + 1))
    dc = np.bincount(det_labels, minlength=ncls)
    oc = np.bincount(labels, minlength=ncls)
    active = np.where((dc > 0) & (oc > 0))[0]
    if len(active) > MAX_SLOTS or (oc[active] > 128).any():
        return None
    order = active[np.argsort(-dc[active], kind="stable")]
    # split the biggest classes' detections in half across the spare
    # slots (objects duplicated; the host maxes the piece q-columns)
    pieces = []
    nsplit = min(MAX_SLOTS - len(order), len(order))
    for r, cls in enumerate(order):
        nd = int(dc[cls])
        if r < nsplit and nd > 1:
            h = nd // 2
            pieces.append((int(cls), int(oc[cls]), h, 0))
            pieces.append((int(cls), int(oc[cls]), nd - h, h))
        else:
            pieces.append((int(cls), int(oc[cls]), nd, 0))
    pieces.sort(key=lambda p: -p[2])
    assign = [[None] * S_SLOTS for _ in range(N_CORES)]
    fds = [16] * S_SLOTS
    for r, p in enumerate(pieces):
        c, s = r % N_CORES, r // N_CORES
        assign[c][s] = p
        fds[s] = max(fds[s], p[2])
    fds = tuple(-(-f // 8) * 8 for f in fds)
    return fds, assign


def _prep_fast_inputs(det_boxes, det_labels, boxes, labels, fds, assign):
    det_boxes = det_boxes.astype(np.float32)
    boxes = boxes.astype(np.float32)
    ftot = sum(fds)
    xoff = [2 * sum(fds[:s]) for s in range(S_SLOTS)]
    yoff = [2 * ftot + 2 * sum(fds[:s]) for s in range(S_SLOTS)]
    aoff = [sum(fds[:s]) for s in range(S_SLOTS)]

    det_order = np.argsort(det_labels, kind="stable")
    obj_order = np.argsort(labels, kind="stable")
    ncls = int(max(N_CLASSES, det_labels.max() + 1, labels.max() + 1))
    dc = np.bincount(det_labels, minlength=ncls)
    oc = np.bincount(labels, minlength=ncls)
    det_off = np.concatenate([[0], np.cumsum(dc)])
    obj_off = np.concatenate([[0], np.cumsum(oc)])

    in_maps = []
    for c in range(N_CORES):
        drow = np.empty(4 * ftot, dtype=np.float16)
        marow = np.ones((2, ftot + 128 * S_SLOTS), dtype=np.float16)
        marow[1, :ftot] = PAD_AD
        objs = np.zeros((128, S_SLOTS * 8), dtype=np.float32)
        for s in range(S_SLOTS):
            f = fds[s]
            ox, oy = xoff[s], yoff[s]
            drow[ox + 0 * f:ox + 1 * f] = PAD_LO   # dx1
            drow[ox + 1 * f:ox + 2 * f] = PAD_HI   # dx2
            drow[oy + 0 * f:oy + 1 * f] = PAD_LO   # dy1
            drow[oy + 1 * f:oy + 2 * f] = PAD_HI   # dy2
            objs[:, 8 * s + 3] = 1.0   # benign pad box (0,0,0,1)
            objs[:, 8 * s + 4] = 1.0   # pad object area
            a = assign[c][s]
            if a is None:
                continue
            cls, no, nd, dlo = a
            dsel = det_order[det_off[cls] + dlo:det_off[cls] + dlo + nd]
            osel = obj_order[obj_off[cls]:obj_off[cls + 1]]
            d16 = det_boxes[dsel].astype(np.float16)   # (x1,y1,x2,y2)
            drow[ox + 0 * f:ox + 0 * f + nd] = d16[:, 0]   # dx1
            drow[ox + 1 * f:ox + 1 * f + nd] = d16[:, 2]   # dx2
            drow[oy + 0 * f:oy + 0 * f + nd] = d16[:, 1]   # dy1
            drow[oy + 1 * f:oy + 1 * f + nd] = d16[:, 3]   # dy2
            marow[1, aoff[s]:aoff[s] + nd] = (            # ad
                (d16[:, 2].astype(np.float32) - d16[:, 0]) *
                (d16[:, 3].astype(np.float32) - d16[:, 1])
            ).astype(np.float16)
            ob = boxes[osel]
            o16 = ob.astype(np.float16).astype(np.float32)
            objs[:no, 8 * s + 0] = o16[:, 0]
            objs[:no, 8 * s + 1] = o16[:, 2]
            objs[:no, 8 * s + 2] = o16[:, 1]
            objs[:no, 8 * s + 3] = o16[:, 3]
            objs[:no, 8 * s + 4] = ((o16[:, 2] - o16[:, 0]) *
                                    (o16[:, 3] - o16[:, 1]))
            marow[0, ftot + 128 * s:ftot + 128 * s + no] = (
                (o16[:, 2] - o16[:, 0]) * (o16[:, 3] - o16[:, 1])
            ).astype(np.float16)
        in_maps.append({"drow": drow.reshape(1, 4 * ftot), "marow": marow,
                        "objs": objs})
    return in_maps


def _fast_loss(results, assign):
    best = {}
    for c in range(N_CORES):
        q = results[c]["qout"]
        for s in range(S_SLOTS):
            a = assign[c][s]
            if a is None:
                continue
            cls, no, _, _ = a
            qs = np.clip(q[:no, s].astype(np.float64), 0.0, None)
            if cls in best:
                best[cls] = np.maximum(best[cls], qs)
            else:
                best[cls] = qs
    num = 0.0
    npos = 0
    for cls, qs in best.items():
        iou = qs / (1.0 - qs)
        num += float(np.sum(1.0 - iou))
        npos += len(qs)
    return np.asarray(np.float32(num / npos))


# ---------------------------------------------------------------------------
# dense fallback (any input)

def _build_dense():
    """Dense kernel: all dets (on partitions) x this core's objects (free)."""
    nc = bacc.Bacc("TRN2", target_bir_lowering=False, debug=False,
                   num_devices=N_CORES)
    F = OBJ_PER_CORE

    detp_d = nc.dram_tensor("detp", [128, 5, T_DET], F32, kind="ExternalInput")
    objr_d = nc.dram_tensor("objr", [5, F], F32, kind="ExternalInput")
    part_d = nc.dram_tensor("partial", [1, 2], F32, kind="ExternalOutput")

    with tile.TileContext(nc) as tc, ExitStack() as ctx:
        cpool = ctx.enter_context(tc.tile_pool(name="const", bufs=1))
        wpool = ctx.enter_context(tc.tile_pool(name="work", bufs=3))

        detp = cpool.tile([128, 5, T_DET], F32)
        nc.sync.dma_start(detp[:], detp_d[:])
        names = ["ox1", "oy1", "ox2", "oy2", "olab"]
        ob = {}
        for i, nm in enumerate(names):
            row = cpool.tile([1, F], F32, tag=f"r_{nm}")
            nc.sync.dma_start(row[:], objr_d[i:i + 1, :])
            t = cpool.tile([128, F], F32, tag=f"b_{nm}")
            nc.gpsimd.partition_broadcast(t[:], row[:], channels=128)
            ob[nm] = t

        aob = cpool.tile([128, F], F32)
        wob = wpool.tile([128, F], F32, tag="wob")
        nc.vector.tensor_tensor(wob[:], ob["ox2"][:], ob["ox1"][:], OP.subtract)
        hob = wpool.tile([128, F], F32, tag="hob")
        nc.vector.tensor_tensor(hob[:], ob["oy2"][:], ob["oy1"][:], OP.subtract)
        nc.vector.tensor_tensor(aob[:], wob[:], hob[:], OP.mult)

        ad = cpool.tile([128, T_DET], F32)
        wd = wpool.tile([128, T_DET], F32, tag="wd")
        nc.vector.tensor_tensor(wd[:], detp[:, 2, :], detp[:, 0, :], OP.subtract)
        hd = wpool.tile([128, T_DET], F32, tag="hd")
        nc.vector.tensor_tensor(hd[:], detp[:, 3, :], detp[:, 1, :], OP.subtract)
        nc.vector.tensor_tensor(ad[:], wd[:], hd[:], OP.mult)

        bmax = cpool.tile([128, F], F32)
        nc.vector.memset(bmax[:], 0.0)
        hm = cpool.tile([128, F], F32)
        nc.vector.memset(hm[:], 0.0)

        for t in range(T_DET):
            dx1 = detp[:, 0, t:t + 1]
            dy1 = detp[:, 1, t:t + 1]
            dx2 = detp[:, 2, t:t + 1]
            dy2 = detp[:, 3, t:t + 1]
            dlab = detp[:, 4, t:t + 1]
            adt = ad[:, t:t + 1]

            mnx = wpool.tile([128, F], F32, tag="mnx")
            nc.vector.tensor_scalar(mnx[:], ob["ox2"][:], dx2, None, op0=OP.min)
            mxx = wpool.tile([128, F], F32, tag="mxx")
            nc.vector.tensor_scalar(mxx[:], ob["ox1"][:], dx1, None, op0=OP.max)
            wx = wpool.tile([128, F], F32, tag="wx")
            nc.vector.tensor_tensor(wx[:], mnx[:], mxx[:], OP.subtract)
            wxr = wpool.tile([128, F], F32, tag="wxr")
            nc.vector.tensor_scalar(wxr[:], wx[:], 0.0, None, op0=OP.max)

            mny = wpool.tile([128, F], F32, tag="mny")
            nc.vector.tensor_scalar(mny[:], ob["oy2"][:], dy2, None, op0=OP.min)
            mxy = wpool.tile([128, F], F32, tag="mxy")
            nc.vector.tensor_scalar(mxy[:], ob["oy1"][:], dy1, None, op0=OP.max)
            wy = wpool.tile([128, F], F32, tag="wy")
            nc.vector.tensor_tensor(wy[:], mny[:], mxy[:], OP.subtract)
            wyr = wpool.tile([128, F], F32, tag="wyr")
            nc.vector.tensor_scalar(wyr[:], wy[:], 0.0, None, op0=OP.max)

            inter = wpool.tile([128, F], F32, tag="inter")
            nc.vector.tensor_tensor(inter[:], wxr[:], wyr[:], OP.mult)
            sab = wpool.tile([128, F], F32, tag="sab")
            nc.vector.tensor_scalar(sab[:], aob[:], adt, None, op0=OP.add)
            denom = wpool.tile([128, F], F32, tag="denom")
            nc.vector.tensor_tensor(denom[:], sab[:], inter[:], OP.subtract)
            rec = wpool.tile([128, F], F32, tag="rec")
            nc.vector.reciprocal(rec[:], denom[:])
            iou = wpool.tile([128, F], F32, tag="iou")
            nc.vector.tensor_tensor(iou[:], inter[:], rec[:], OP.mult)

            eq = wpool.tile([128, F], F32, tag="eq")
            nc.vector.tensor_scalar(eq[:], ob["olab"][:], dlab, None,
                                    op0=OP.is_equal)
            miou = wpool.tile([128, F], F32, tag="miou")
            nc.vector.tensor_tensor(miou[:], iou[:], eq[:], OP.mult)

            nc.vector.tensor_tensor(bmax[:], bmax[:], miou[:], OP.max)
            nc.vector.tensor_tensor(hm[:], hm[:], eq[:], OP.max)

        bred = cpool.tile([128, F], F32)
        nc.gpsimd.partition_all_reduce(bred[:], bmax[:], 128, ReduceOp.max)
        hred = cpool.tile([128, F], F32)
        nc.gpsimd.partition_all_reduce(hred[:], hm[:], 128, ReduceOp.max)

        c1 = wpool.tile([1, F], F32, tag="c1")
        nc.vector.tensor_scalar(c1[:], bred[0:1, :], -1.0, 1.0,
                                op0=OP.mult, op1=OP.add)
        c2 = wpool.tile([1, F], F32, tag="c2")
        nc.vector.tensor_tensor(c2[:], c1[:], hred[0:1, :], OP.mult)

        outt = wpool.tile([1, 2], F32, tag="outt")
        nc.vector.tensor_reduce(outt[:, 0:1], c2[:], AX.X, OP.add)
        nc.vector.tensor_reduce(outt[:, 1:2], hred[0:1, :], AX.X, OP.add)
        nc.sync.dma_start(part_d[:], outt[:])

    nc.compile()
    return nc


def _prep_dense_inputs(det_boxes, det_labels, boxes, labels):
    det = np.full((DET_PAD, 5), -5.0, dtype=np.float32)
    det[:N_DET, 0:4] = det_boxes.astype(np.float32)
    det[:N_DET, 4] = det_labels.astype(np.float32)
    det[N_DET:, 4] = -1.0
    detp = np.ascontiguousarray(
        det.reshape(T_DET, 128, 5).transpose(1, 2, 0))

    in_maps = []
    for c in range(N_CORES):
        sl = slice(c * OBJ_PER_CORE, (c + 1) * OBJ_PER_CORE)
        objr = np.empty((5, OBJ_PER_CORE), dtype=np.float32)
        objr[0:4, :] = boxes[sl].astype(np.float32).T
        objr[4, :] = labels[sl].astype(np.float32)
        in_maps.append({"detp": detp, "objr": objr})
    return in_maps


_CACHE = {}


def _get_dense():
    if "dense" not in _CACHE:
        _CACHE["dense"] = _build_dense()
    return _CACHE["dense"]


def _get_fast(fds):
    key = f"fast{fds}"
    if key not in _CACHE:
        _CACHE[key] = _build_fast(fds)
    return _CACHE[key]


def kernel(det_boxes, det_scores, det_labels, boxes, labels):
    det_boxes = np.asarray(det_boxes)
    det_labels = np.asarray(det_labels)
    boxes = np.asarray(boxes)
    labels = np.asarray(labels)

    plan = _assign_classes(det_labels, labels)
    if plan is not None:
        fds, assign = plan
        in_maps = _prep_fast_inputs(det_boxes, det_labels, boxes, labels,
                                    fds, assign)
        res = run_bass_kernel_spmd(_get_fast(fds), in_maps,
                                   list(range(N_CORES)))
        return _fast_loss(res.results, assign)

    in_maps = _prep_dense_inputs(det_boxes, det_labels, boxes, labels)
    res = run_bass_kernel_spmd(_get_dense(), in_maps, list(range(N_CORES)))
    tot = np.zeros(2, dtype=np.float32)
    for c in range(N_CORES):
        p = res.results[c]["partial"]
        tot += p.sum(axis=0, dtype=np.float32) if p.shape[0] > 1 else p[0]
    return np.asarray(np.float32(tot[0] / tot[1]))

